# revision 1
# baseline (speedup 1.0000x reference)
"""Multi-head attention (B=2, S=2048, D=1024, H=16, HD=64) on 8 trn2 cores.

Sharding: core c handles batch b = c // 4 and the 4 heads
[4*(c%4), 4*(c%4)+4)  (tensor-parallel split of the Wq/Wk/Wv column dim,
data-parallel over batch).  Each core computes its heads' full SxS
attention locally; no collectives.

Per-core algorithm (all matmuls in fp32r on the PE):
  1. X^T built in SBUF via PE transposes ([d, s] layout, d on partitions).
  2. Q^T, K^T computed as psum = W[k, dout].T-chain over k  -> [dout, s].
     V computed in natural [s, dout] layout (lhsT = X^T tiles), stored
     with a per-head all-ones 65th column for the fused softmax-sum.
  3. Per head, per 1024-wide m-chunk of queries:
       scoresT[t, m] = K^T_h[:, t-tile].T @ Q^T_h   (K = HD = 64)
       expT = exp(scoresT / 8)        (ACT, reads psum directly)
       av[65, m]  += [V_h | 1].T @ expT   (accumulated over t-tiles;
                                           row 64 = softmax denominator)
     then transpose av back to [m, 65] via PE, divide rows by the
     denominator (DVE reciprocal + per-partition scalar multiply) and DMA
     the [128, 64] result blocks to the output.
"""

import os
from contextlib import ExitStack, nullcontext

import numpy as np

import concourse.bacc as bacc
import concourse.mybir as mybir
import concourse.tile as tile
from concourse.bass_utils import run_bass_kernel_spmd
from concourse.masks import make_identity

B, S, D = 2, 2048, 1024
H, HD = 16, 64
NCORES = 8
HPC = H * B // NCORES          # heads per core = 4
HG = HPC * HD                  # per-core projection width = 256
P = 128
KT = D // P                    # 8 contraction tiles
ST = S // P                    # 16 sequence tiles
MC = 1024                      # m-chunk width for the attention loop
NMC = S // MC
VW = HD + 1                    # V columns per head incl. ones column = 65

F32 = mybir.dt.float32
BF16 = mybir.dt.bfloat16
F32R = mybir.dt.float32r
EXP = mybir.ActivationFunctionType.Exp


def _r(ap):
    return ap.bitcast(F32R)


def build_nc(reps=1):
    nc = bacc.Bacc(
        "TRN2", target_bir_lowering=False, debug=False, num_devices=NCORES
    )
    x = nc.dram_tensor("x", [S, D], F32, kind="ExternalInput")
    wq = nc.dram_tensor("wq", [D, HG], F32, kind="ExternalInput")
    wk = nc.dram_tensor("wk", [D, HG], F32, kind="ExternalInput")
    wv = nc.dram_tensor("wv", [D, HG], F32, kind="ExternalInput")
    out = nc.dram_tensor("out", [S, HG], F32, kind="ExternalOutput")

    with tile.TileContext(nc) as tc, ExitStack() as ctx:
        big = ctx.enter_context(tc.tile_pool(name="big", bufs=1))
        xst = ctx.enter_context(tc.tile_pool(name="xst", bufs=4))
        expp = ctx.enter_context(tc.tile_pool(name="expp", bufs=4))
        osbp = ctx.enter_context(tc.tile_pool(name="osbp", bufs=2))
        outp = ctx.enter_context(tc.tile_pool(name="outp", bufs=4))
        recp = ctx.enter_context(tc.tile_pool(name="recp", bufs=4))
        # PSUM budget (8 banks): pp_ss 3 slots x 2 banks = 6 (shared tag for
        # scores/proj/V/transposes -- 3 slots keeps psum slot-reuse waits two
        # steps back so they never drain the PE pipeline), pp_av 1x[65,1024]=2.
        pp_ss = ctx.enter_context(tc.tile_pool(name="pp_ss", bufs=3, space="PSUM"))
        pp_av = ctx.enter_context(tc.tile_pool(name="pp_av", bufs=1, space="PSUM"))

        rep_ctx = tc.For_i(0, reps, 1) if reps > 1 else nullcontext()
        with rep_ctx:
            ident = big.tile([P, P], F32)
            make_identity(nc, ident[:])
            identr = big.tile([P, P], F32)
            nc.vector.tensor_copy(_r(identr[:]), ident[:])

            # ---- persistent SBUF tensors ----
            XT = big.tile([P, KT * S], F32)        # X^T: col(kt, s) = kt*S + s
            WQs = big.tile([P, KT * HG], F32)      # col(kt, j) = kt*HG + j
            WKs = big.tile([P, KT * HG], F32)
            WVs = big.tile([P, KT * HG], F32)
            QT = big.tile([P, 2 * S], F32)         # col(jtile, m) = jtile*S + m
            KTt = big.tile([P, 2 * S], F32)
            Vn = big.tile([P, ST * HPC * VW], BF16)  # col(st, h, e) = st*HPC*VW + h*VW + e

            # ---- load weights (one DMA per weight matrix) ----
            for Wt, w in ((WQs, wq), (WKs, wk), (WVs, wv)):
                nc.sync.dma_start(
                    _r(Wt[:].rearrange("p (k n) -> p k n", n=HG)),
                    _r(w[:].rearrange("(k p) n -> p k n", p=P)),
                )

            # ones columns of Vn (col 64 of each head block). memset can't write
            # f32r, so memset a small f32 tile and round via tensor_copy.
            ones_ap = Vn[:].rearrange("p (s h e) -> p s h e", h=HPC, e=VW)[:, :, :, HD:VW]
            ones_stage = big.tile([P, ST * HPC], F32)
            nc.vector.memset(ones_stage[:], 1.0)
            nc.vector.tensor_copy(
                ones_ap,
                ones_stage[:].rearrange("p (s h e) -> p s h e", h=HPC, e=1),
            )

            # ---- load X (two s-tiles per DMA) and transpose into XT ----
            XT3 = XT[:].rearrange("p (k s) -> p k s", s=S)
            for sp in range(ST // 2):
                xs = xst.tile([P, 2 * D], F32)
                nc.sync.dma_start(
                    _r(xs[:].rearrange("p (t d) -> p t d", d=D)),
                    _r(x[sp * 2 * P:(sp + 1) * 2 * P, :].rearrange("(t p) d -> p t d", p=P)),
                )
                for tt in range(2):
                    st = sp * 2 + tt
                    for g in range(2):
                        pt = pp_ss.tile([P, 512], F32, tag="ps")
                        for j in range(4):
                            kt = g * 4 + j
                            nc.tensor.transpose(
                                _r(pt[:, j * P:(j + 1) * P]),
                                _r(xs[:, tt * D + kt * P: tt * D + (kt + 1) * P]),
                                _r(identr[:]),
                            )
                        dst = XT3[:, g * 4:(g + 1) * 4, st * P:(st + 1) * P]
                        src = pt[:].rearrange("p (k s) -> p k s", s=P)
                        nc.vector.tensor_copy(_r(dst), src)

            # ---- projections / attention, emitted interleaved so the
            # scheduler can fill PE gaps of the ACT-bound attention phase with
            # the later head-group's projection matmuls ----
            Vn4 = Vn[:].rearrange("p (s h e) -> p s h e", h=HPC, e=VW)

            def emit_proj_qk(Wt, Ot, j):
                for nn in range(4):
                    pt = pp_ss.tile([P, 512], F32, tag="ps", name="pt")
                    for kt in range(KT):
                        nc.tensor.matmul(
                            pt[:],
                            _r(Wt[:, kt * HG + j * P: kt * HG + (j + 1) * P]),
                            _r(XT[:, kt * S + nn * 512: kt * S + (nn + 1) * 512]),
                            start=(kt == 0),
                            stop=(kt == KT - 1),
                        )
                    nc.vector.tensor_copy(
                        _r(Ot[:, j * S + nn * 512: j * S + (nn + 1) * 512]), pt[:]
                    )

            def emit_v():
                for st in range(ST):
                    pt = pp_ss.tile([P, HG], F32, tag="ps", name="pt")
                    for kt in range(KT):
                        nc.tensor.matmul(
                            pt[:],
                            _r(XT[:, kt * S + st * P: kt * S + (st + 1) * P]),
                            _r(WVs[:, kt * HG:(kt + 1) * HG]),
                            start=(kt == 0),
                            stop=(kt == KT - 1),
                        )
                    nc.vector.tensor_copy(
                        Vn4[:, st, :, 0:HD], pt[:].rearrange("p (h e) -> p h e", e=HD)
                    )

            def emit_attn(h):
                j = h // 2
                prow = (h % 2) * 64
                qh = QT[prow:prow + 64, j * S:(j + 1) * S]
                kh = KTt[prow:prow + 64, j * S:(j + 1) * S]
                def mm_scores(t, mc):
                    ps = pp_ss.tile([P, MC], F32, tag="ps", name="ps")
                    for hf in range(MC // 512):
                        nc.tensor.matmul(
                            ps[:, hf * 512:(hf + 1) * 512],
                            _r(kh[:, t * P:(t + 1) * P]),
                            _r(qh[:, mc * MC + hf * 512: mc * MC + (hf + 1) * 512]),
                            start=True,
                            stop=True,
                        )
                    return ps

                for mc in range(NMC):
                    av = pp_av.tile([VW, MC], F32, tag="av", name="av")
                    # software pipeline: emit MM_s(t+1) BEFORE exp(t)/MM_av(t)
                    # so the PE never sits behind the ACT exp in program order.
                    ps = mm_scores(0, mc)
                    for t in range(ST):
                        ps_cur = ps
                        if t + 1 < ST:
                            ps = mm_scores(t + 1, mc)
                        ex = expp.tile([P, MC], BF16, tag="ex", name="ex")
                        nc.scalar.activation(ex[:], ps_cur[:], EXP, scale=1.0 / np.sqrt(HD))
                        for hf in range(MC // 512):
                            nc.tensor.matmul(
                                av[:, hf * 512:(hf + 1) * 512],
                                Vn[:, t * HPC * VW + h * VW: t * HPC * VW + (h + 1) * VW],
                                ex[:, hf * 512:(hf + 1) * 512],
                                start=(t == 0),
                                stop=(t == ST - 1),
                            )
                    # evacuate, transpose back, normalize, store (one DMA per
                    # (head, m-chunk))
                    osb = osbp.tile([VW, MC], F32, tag="osb", name="osb")
                    nc.vector.tensor_copy(osb[:], av[:])
                    ot = outp.tile([P, (MC // P) * HD], F32, tag="ot", name="ot")
                    for ms in range(MC // P):
                        po = pp_ss.tile([P, 512], F32, tag="ps", name="po")
                        pot = po[:, 0:VW]
                        nc.tensor.transpose(
                            pot, osb[0:VW, ms * P:(ms + 1) * P], ident[0:VW, 0:VW]
                        )
                        rec = recp.tile([P, 1], F32, tag="rec", name="rec")
                        nc.vector.reciprocal(rec[:], pot[:, HD:VW])
                        nc.vector.tensor_scalar_mul(
                            ot[:, ms * HD:(ms + 1) * HD], pot[:, 0:HD], rec[:]
                        )
                    row0 = mc * MC
                    nc.sync.dma_start(
                        out[row0:row0 + MC, h * HD:(h + 1) * HD].rearrange(
                            "(t p) e -> p t e", p=P
                        ),
                        ot[:].rearrange("p (t e) -> p t e", e=HD),
                    )

            emit_proj_qk(WQs, QT, 0)
            emit_proj_qk(WKs, KTt, 0)
            emit_v()
            emit_attn(0)
            emit_proj_qk(WQs, QT, 1)
            emit_proj_qk(WKs, KTt, 1)
            emit_attn(1)
            emit_attn(2)
            emit_attn(3)

    nc.compile()
    return nc


_NC = None


def _get_nc():
    global _NC
    if _NC is None:
        _NC = build_nc()
    return _NC


def _shard_inputs(inputs, Wq, Wk, Wv):
    inputs = np.ascontiguousarray(np.asarray(inputs, dtype=np.float32))
    Wq = np.asarray(Wq, dtype=np.float32)
    Wk = np.asarray(Wk, dtype=np.float32)
    Wv = np.asarray(Wv, dtype=np.float32)
    in_maps = []
    for c in range(NCORES):
        b, g = c // (NCORES // B), c % (NCORES // B)
        sl = slice(g * HG, (g + 1) * HG)
        in_maps.append(
            {
                "x": inputs[b],
                "wq": np.ascontiguousarray(Wq[:, sl]),
                "wk": np.ascontiguousarray(Wk[:, sl]),
                "wv": np.ascontiguousarray(Wv[:, sl]),
            }
        )
    return in_maps


def _gather(results):
    out = np.empty((B, S, H * HD), dtype=np.float32)
    for c in range(NCORES):
        b, g = c // (NCORES // B), c % (NCORES // B)
        out[b, :, g * HG:(g + 1) * HG] = results[c]["out"]
    return out


def kernel(inputs, Wq, Wk, Wv):
    nc = _get_nc()
    in_maps = _shard_inputs(inputs, Wq, Wk, Wv)
    res = run_bass_kernel_spmd(nc, in_maps, core_ids=list(range(NCORES)))
    return _gather(res.results)



# revision 10
# speedup vs baseline: 1.7081x; 1.7081x over previous
"""Multi-head attention (B=2, S=2048, D=1024, H=16, HD=64) on 8 trn2 cores.

Sharding: core c handles batch b = c // 4 and the 4 heads
[4*(c%4), 4*(c%4)+4)  (tensor-parallel split of the Wq/Wk/Wv column dim,
data-parallel over batch).  Each core computes its heads' full SxS
attention locally; no collectives.

Per-core algorithm (all matmuls in fp32r on the PE):
  1. X^T built in SBUF via PE transposes ([d, s] layout, d on partitions).
  2. Q^T, K^T computed as psum = W[k, dout].T-chain over k  -> [dout, s].
     V computed in natural [s, dout] layout (lhsT = X^T tiles), stored
     with a per-head all-ones 65th column for the fused softmax-sum.
  3. Attention runs per head-PAIR: the two heads of pair j live at
     partition rows 0:64 / 64:128 of QT/KTt block j, so their score
     matmuls (contraction K=HD=64) issue back-to-back into disjoint
     PE row-groups and execute concurrently (2x effective rate).
     Per (pair, 512-wide m-chunk, t-tile):
       ps[128, 1024] = [scoresT_h0 | scoresT_h1]     (2 psum banks)
       ex = exp(ps / 8) in ONE [128,1024] instr -- ACT for most t,
            DVE int16-Schraudolph (bitcast bf16) for a few t to keep
            the scalar engine off the critical path.
       av_h[65, 512] += [V_h | 1].T @ ex_h           (row 64 = denom)
     then per head: transpose av back via PE, divide rows by the
     denominator (DVE reciprocal + per-partition scalar multiply) and
     DMA the [128, 4*64] result block to the output.
"""

import os
from contextlib import ExitStack, nullcontext

import numpy as np

import concourse.bacc as bacc
import concourse.mybir as mybir
import concourse.tile as tile
from concourse.bass_utils import run_bass_kernel_spmd
from concourse.masks import make_identity

B, S, D = 2, 2048, 1024
H, HD = 16, 64
NCORES = 8
HPC = H * B // NCORES          # heads per core = 4
HG = HPC * HD                  # per-core projection width = 256
P = 128
KT = D // P                    # 8 contraction tiles
ST = S // P                    # 16 sequence tiles
MC = 512                       # m-chunk width for the attention loop
NMC = S // MC                  # 4
VW = HD + 1                    # V columns per head incl. ones column = 65

F32 = mybir.dt.float32
BF16 = mybir.dt.bfloat16
F32R = mybir.dt.float32r
I16 = mybir.dt.int16
EXP = mybir.ActivationFunctionType.Exp
MULT = mybir.AluOpType.mult
ADD = mybir.AluOpType.add

# int16 Schraudolph exp: exp(s) ~= bitcast_bf16(round(s*2^7/ln2 + (127-c)*2^7))
# with s = raw_score/8 folded into the multiplier. c ~ 0.045 minimizes RMS
# relative error (~2%); end-to-end softmax output err ~1e-3 per offloaded
# 1/16 of tiles (validated in numpy against exact softmax).
SCH_A = (2.0 ** 7) / np.log(2.0) / 8.0
SCH_B = (127.0 - 0.045) * (2.0 ** 7) + 0.25
# t-tiles whose exp runs on DVE instead of ACT (4 of 16); keep DVE off the
# chain head (t=0) and tail (t=15) so its queue lag never gates av stop.
DVE_T = (2, 5, 9, 12)


def _r(ap):
    return ap.bitcast(F32R)


def build_nc(reps=1):
    nc = bacc.Bacc(
        "TRN2", target_bir_lowering=False, debug=False, num_devices=NCORES
    )
    x = nc.dram_tensor("x", [S, D], F32, kind="ExternalInput")
    wq = nc.dram_tensor("wq", [D, HG], F32, kind="ExternalInput")
    wk = nc.dram_tensor("wk", [D, HG], F32, kind="ExternalInput")
    wv = nc.dram_tensor("wv", [D, HG], F32, kind="ExternalInput")
    out = nc.dram_tensor("out", [S, HG], F32, kind="ExternalOutput")

    with tile.TileContext(nc) as tc, ExitStack() as ctx:
        big = ctx.enter_context(tc.tile_pool(name="big", bufs=1))
        xst = ctx.enter_context(tc.tile_pool(name="xst", bufs=4))
        expp = ctx.enter_context(tc.tile_pool(name="expp", bufs=4))
        osbp = ctx.enter_context(tc.tile_pool(name="osbp", bufs=4))
        outp = ctx.enter_context(tc.tile_pool(name="outp", bufs=4))
        recp = ctx.enter_context(tc.tile_pool(name="recp", bufs=8))
        # PSUM budget (8 banks): pp_ss 2 slots x [128,1024] = 4 banks
        # (shared tag for scores/proj/V), pp_po 2 x [128,512] = 2 banks
        # (output transposes get their own pool so they never steal a
        # scores slot at chunk boundaries), pp_av 2 x [65,512] = 2.
        pp_ss = ctx.enter_context(tc.tile_pool(name="pp_ss", bufs=2, space="PSUM"))
        pp_po = ctx.enter_context(tc.tile_pool(name="pp_po", bufs=2, space="PSUM"))
        pp_av = ctx.enter_context(tc.tile_pool(name="pp_av", bufs=2, space="PSUM"))

        rep_ctx = tc.For_i(0, reps, 1) if reps > 1 else nullcontext()
        with rep_ctx:
            ident = big.tile([P, P], F32)
            make_identity(nc, ident[:])
            identr = big.tile([P, P], F32)
            nc.vector.tensor_copy(_r(identr[:]), ident[:])

            # ---- persistent SBUF tensors ----
            XT = big.tile([P, KT * S], F32)        # X^T: col(kt, s) = kt*S + s
            WQs = big.tile([P, KT * HG], F32)      # col(kt, j) = kt*HG + j
            WKs = big.tile([P, KT * HG], F32)
            WVs = big.tile([P, KT * HG], F32)
            QT = big.tile([P, 2 * S], F32)         # col(jtile, m) = jtile*S + m
            KTt = big.tile([P, 2 * S], F32)
            Vn = big.tile([P, ST * HPC * VW], BF16)  # col(st, h, e) = st*HPC*VW + h*VW + e

            # ---- load weights (one DMA per weight matrix), each on its own
            # engine queue so they never serialize behind the X loads on SP's
            # queue (the first PE transpose gates on the first X chunk) ----
            for eng, Wt, w in (
                (nc.scalar, WQs, wq),
                (nc.gpsimd, WKs, wk),
                (nc.gpsimd, WVs, wv),
            ):
                eng.dma_start(
                    _r(Wt[:].rearrange("p (k n) -> p k n", n=HG)),
                    _r(w[:].rearrange("(k p) n -> p k n", p=P)),
                )

            # ones columns of Vn (col 64 of each head block). memset can't write
            # f32r, so memset a small f32 tile and round via tensor_copy.
            ones_ap = Vn[:].rearrange("p (s h e) -> p s h e", h=HPC, e=VW)[:, :, :, HD:VW]
            ones_stage = big.tile([P, ST * HPC], F32)
            nc.vector.memset(ones_stage[:], 1.0)
            nc.vector.tensor_copy(
                ones_ap,
                ones_stage[:].rearrange("p (s h e) -> p s h e", h=HPC, e=1),
            )

            # ---- load X (two s-tiles per DMA) and transpose into XT ----
            XT3 = XT[:].rearrange("p (k s) -> p k s", s=S)
            for sp in range(ST // 2):
                xs = xst.tile([P, 2 * D], F32)
                nc.sync.dma_start(
                    _r(xs[:].rearrange("p (t d) -> p t d", d=D)),
                    _r(x[sp * 2 * P:(sp + 1) * 2 * P, :].rearrange("(t p) d -> p t d", p=P)),
                )
                for tt in range(2):
                    st = sp * 2 + tt
                    for g in range(2):
                        pt = pp_ss.tile([P, 1024], F32, tag="ps")
                        for j in range(4):
                            kt = g * 4 + j
                            nc.tensor.transpose(
                                _r(pt[:, j * P:(j + 1) * P]),
                                _r(xs[:, tt * D + kt * P: tt * D + (kt + 1) * P]),
                                _r(identr[:]),
                            )
                        dst = XT3[:, g * 4:(g + 1) * 4, st * P:(st + 1) * P]
                        src = pt[:, 0:512].rearrange("p (k s) -> p k s", s=P)
                        # alternate psum evacuation between DVE and ACT --
                        # ACT is otherwise idle during the X^T build.
                        if (st * 2 + g) % 2 == 0:
                            nc.vector.tensor_copy(_r(dst), src)
                        else:
                            nc.scalar.copy(_r(dst), src)

            # ---- projections / attention, emitted interleaved so the
            # scheduler can fill PE gaps of the ACT-bound attention phase with
            # the later head-group's projection matmuls ----
            Vn4 = Vn[:].rearrange("p (s h e) -> p s h e", h=HPC, e=VW)

            def emit_proj_qk(Wt, Ot, j):
                for nn in range(4):
                    pt = pp_ss.tile([P, 1024], F32, tag="ps", name="pt")
                    for kt in range(KT):
                        nc.tensor.matmul(
                            pt[:, 0:512],
                            _r(Wt[:, kt * HG + j * P: kt * HG + (j + 1) * P]),
                            _r(XT[:, kt * S + nn * 512: kt * S + (nn + 1) * 512]),
                            start=(kt == 0),
                            stop=(kt == KT - 1),
                        )
                    if nn % 2 == 0:
                        nc.vector.tensor_copy(
                            _r(Ot[:, j * S + nn * 512: j * S + (nn + 1) * 512]),
                            pt[:, 0:512],
                        )
                    else:
                        nc.scalar.copy(
                            _r(Ot[:, j * S + nn * 512: j * S + (nn + 1) * 512]),
                            pt[:, 0:512],
                        )

            def emit_v():
                for st in range(ST):
                    pt = pp_ss.tile([P, 1024], F32, tag="ps", name="pt")
                    for kt in range(KT):
                        nc.tensor.matmul(
                            pt[:, 0:HG],
                            _r(XT[:, kt * S + st * P: kt * S + (st + 1) * P]),
                            _r(WVs[:, kt * HG:(kt + 1) * HG]),
                            start=(kt == 0),
                            stop=(kt == KT - 1),
                        )
                    if st % 2 == 0:
                        nc.vector.tensor_copy(
                            Vn4[:, st, :, 0:HD],
                            pt[:, 0:HG].rearrange("p (h e) -> p h e", e=HD),
                        )
                    else:
                        nc.scalar.copy(
                            Vn4[:, st, :, 0:HD],
                            pt[:, 0:HG].rearrange("p (h e) -> p h e", e=HD),
                        )

            def emit_attn_pair(j):
                qh = QT[:, j * S:(j + 1) * S]
                kh = KTt[:, j * S:(j + 1) * S]

                def mm_scores(t, mc):
                    # the two heads' score matmuls contract over disjoint
                    # 64-row groups of the PE array -> concurrent execution
                    ps = pp_ss.tile([P, 2 * MC], F32, tag="ps", name="ps")
                    nc.tensor.matmul(
                        ps[:, 0:MC],
                        _r(kh[0:64, t * P:(t + 1) * P]),
                        _r(qh[0:64, mc * MC:(mc + 1) * MC]),
                        start=True,
                        stop=True,
                    )
                    nc.tensor.matmul(
                        ps[:, MC:2 * MC],
                        _r(kh[64:128, t * P:(t + 1) * P]),
                        _r(qh[64:128, mc * MC:(mc + 1) * MC]),
                        start=True,
                        stop=True,
                    )
                    return ps

                for mc in range(NMC):
                    av01 = [
                        pp_av.tile([VW, MC], F32, tag="av", name="av"),
                        pp_av.tile([VW, MC], F32, tag="av", name="av"),
                    ]
                    # software pipeline: emit MM_s(t+1) BEFORE exp(t)/MM_av(t)
                    # so the PE never sits behind the exp in program order.
                    ps = mm_scores(0, mc)
                    for t in range(ST):
                        ps_cur = ps
                        if t + 1 < ST:
                            ps = mm_scores(t + 1, mc)
                        ex = expp.tile([P, 2 * MC], BF16, tag="ex", name="ex")
                        if t in DVE_T:
                            nc.vector.tensor_scalar(
                                ex[:].bitcast(I16), ps_cur[:], SCH_A, SCH_B, MULT, ADD
                            )
                        else:
                            nc.scalar.activation(
                                ex[:], ps_cur[:], EXP, scale=1.0 / np.sqrt(HD)
                            )
                        for hi in range(2):
                            h = 2 * j + hi
                            nc.tensor.matmul(
                                av01[hi][:],
                                Vn[:, t * HPC * VW + h * VW: t * HPC * VW + (h + 1) * VW],
                                ex[:, hi * MC:(hi + 1) * MC],
                                start=(t == 0),
                                stop=(t == ST - 1),
                            )
                    # evacuate, transpose back, normalize, store (one DMA per
                    # (head, m-chunk))
                    for hi in range(2):
                        h = 2 * j + hi
                        osb = osbp.tile([VW, MC], F32, tag="osb", name="osb")
                        nc.vector.tensor_copy(osb[:], av01[hi][:])
                        ot = outp.tile([P, (MC // P) * HD], F32, tag="ot", name="ot")
                        for ms in range(MC // P):
                            po = pp_po.tile([P, 512], F32, tag="po", name="po")
                            pot = po[:, 0:VW]
                            nc.tensor.transpose(
                                pot, osb[0:VW, ms * P:(ms + 1) * P], ident[0:VW, 0:VW]
                            )
                            rec = recp.tile([P, 1], F32, tag="rec", name="rec")
                            nc.vector.reciprocal(rec[:], pot[:, HD:VW])
                            nc.vector.tensor_scalar_mul(
                                ot[:, ms * HD:(ms + 1) * HD], pot[:, 0:HD], rec[:]
                            )
                        row0 = mc * MC
                        nc.sync.dma_start(
                            out[row0:row0 + MC, h * HD:(h + 1) * HD].rearrange(
                                "(t p) e -> p t e", p=P
                            ),
                            ot[:].rearrange("p (t e) -> p t e", e=HD),
                        )

            emit_proj_qk(WQs, QT, 0)
            emit_proj_qk(WKs, KTt, 0)
            emit_v()
            emit_attn_pair(0)
            emit_proj_qk(WQs, QT, 1)
            emit_proj_qk(WKs, KTt, 1)
            emit_attn_pair(1)

    nc.compile()
    return nc


_NC = None


def _get_nc():
    global _NC
    if _NC is None:
        _NC = build_nc()
    return _NC


def _shard_inputs(inputs, Wq, Wk, Wv):
    inputs = np.ascontiguousarray(np.asarray(inputs, dtype=np.float32))
    Wq = np.asarray(Wq, dtype=np.float32)
    Wk = np.asarray(Wk, dtype=np.float32)
    Wv = np.asarray(Wv, dtype=np.float32)
    in_maps = []
    for c in range(NCORES):
        b, g = c // (NCORES // B), c % (NCORES // B)
        sl = slice(g * HG, (g + 1) * HG)
        in_maps.append(
            {
                "x": inputs[b],
                "wq": np.ascontiguousarray(Wq[:, sl]),
                "wk": np.ascontiguousarray(Wk[:, sl]),
                "wv": np.ascontiguousarray(Wv[:, sl]),
            }
        )
    return in_maps


def _gather(results):
    out = np.empty((B, S, H * HD), dtype=np.float32)
    for c in range(NCORES):
        b, g = c // (NCORES // B), c % (NCORES // B)
        out[b, :, g * HG:(g + 1) * HG] = results[c]["out"]
    return out


def kernel(inputs, Wq, Wk, Wv):
    nc = _get_nc()
    in_maps = _shard_inputs(inputs, Wq, Wk, Wv)
    res = run_bass_kernel_spmd(nc, in_maps, core_ids=list(range(NCORES)))
    return _gather(res.results)
